# revision 8
# baseline (speedup 1.0000x reference)
"""Multi-head attention (B=4, S=2048, D=1024, H=16) on 8 trn2 cores.

Sharding: core c handles batch b = c//2 and query-half h = c%2 (1024 query
positions), computing all 16 heads for those queries. No collectives: k/v
work for a batch is duplicated across its 2 cores.

Layout strategy (everything flows transposed, zero on-device transposes):
  host:    xT = x[b].T, wqkvT = Wqkv.T (q cols pre-scaled 1/sqrt(hd)),
           woutT = Wout.T; all cast to bf16.
  stage 1: qT[qdim, qpos], kT[kdim, kpos] (transposed) and v[seq, vdim]
           (natural, with a ones column per head for softmax sums).
  attn:    scoresT[kpos, qpos] = kT.T-slices @ qT-slices on PE; exp via ACT
           PSUM->SBUF (bf16); PV uses v as stationary so out lands
           transposed [hd+1, qpos]; row hd holds the softmax denominators.
           Heads processed in pairs occupying PE row groups 0-63 / 64-127
           so their K=64 matmuls overlap in the array.
  stage 3: natural out[seq, dim] = attn_outT-slices.T @ woutT; f32 to DRAM.
"""

import numpy as np
import ml_dtypes

B, S, DIM, HEADS, HD = 4, 2048, 1024, 16, 64
N_CORES = 8
QP = S // 2          # query positions per core
GD = DIM // 128      # 8 dim chunks
SC = S // 128        # 16 seq chunks
BF16 = ml_dtypes.bfloat16

_CACHE = {}


def _build_program():
    import concourse.mybir as mybir
    import concourse.tile as tile
    from concourse import bacc

    f32 = mybir.dt.float32
    bf16 = mybir.dt.bfloat16
    Exp = mybir.ActivationFunctionType.Exp

    nc = bacc.Bacc("TRN2", target_bir_lowering=False, debug=False,
                   num_devices=N_CORES)
    d_xT = nc.declare_dram_parameter("xT", [DIM, S], bf16, isOutput=False)
    d_xTq = nc.declare_dram_parameter("xTq", [DIM, QP], bf16, isOutput=False)
    d_wqkvT = nc.declare_dram_parameter("wqkvT", [DIM, 3 * DIM], bf16,
                                        isOutput=False)
    d_woutT = nc.declare_dram_parameter("woutT", [DIM, DIM], bf16,
                                        isOutput=False)
    d_out = nc.declare_dram_parameter("out", [QP, DIM], f32, isOutput=True)

    with tile.TileContext(nc) as tc:
        with (
            tc.tile_pool(name="res", bufs=1) as res,
            # PSUM pools shared across all phases: 4 + 4 banks
            tc.tile_pool(name="big", bufs=2, space="PSUM") as bigp,
            tc.tile_pool(name="pvp", bufs=2, space="PSUM") as pvp,
        ):
            # Whole-kernel resident tiles
            qT = res.tile([128, GD, QP], bf16)          # [p, g, qpos]
            kT = res.tile([128, GD, S], bf16)           # [p, g, kpos]
            v_sb = res.tile([128, SC, HEADS, HD + 1], bf16)  # [p, sc, h, d]
            aoT = res.tile([128, GD, QP], bf16)         # attn outT [p, g, qpos]
            woutT = res.tile([128, GD, DIM], bf16)
            ones = res.tile([1, HD], bf16)

            nc.vector.memset(v_sb[:, :, :, HD:HD + 1], 1.0)
            nc.vector.memset(ones[:], 1.0)
            nc.sync.dma_start(
                out=woutT[:],
                in_=d_woutT.ap().rearrange("(g p) n -> p g n", p=128))

            # ---------------- stage 1: qkv projection ----------------
            with (
                tc.tile_pool(name="s1sb", bufs=1) as s1sb,
                tc.tile_pool(name="s1w", bufs=4) as s1w,
            ):
                xT = s1sb.tile([128, GD, S], bf16)
                xTq = s1sb.tile([128, GD, QP], bf16)
                wv0 = s1sb.tile([128, GD, 512], bf16)
                wv1 = s1sb.tile([128, GD, 512], bf16)
                nc.sync.dma_start(
                    out=xT[:], in_=d_xT.ap().rearrange("(g p) s -> p g s", p=128))
                nc.sync.dma_start(
                    out=xTq[:], in_=d_xTq.ap().rearrange("(g p) s -> p g s", p=128))
                for n, wv in enumerate((wv0, wv1)):
                    nc.sync.dma_start(
                        out=wv[:],
                        in_=d_wqkvT.ap()[:, 2 * DIM + n * 512:2 * DIM + (n + 1) * 512]
                        .rearrange("(g p) n -> p g n", p=128))

                # v: natural layout, xT as stationary (first: PV needs it first)
                for sc in range(SC):
                    ps = bigp.tile([128, 1024], f32, tag="big")
                    for n, wv in enumerate((wv0, wv1)):
                        for g in range(GD):
                            nc.tensor.matmul(
                                out=ps[:, n * 512:(n + 1) * 512],
                                lhsT=xT[:, g, sc * 128:(sc + 1) * 128],
                                rhs=wv[:, g, :],
                                start=(g == 0), stop=(g == GD - 1))
                    nc.vector.tensor_copy(out=v_sb[:, sc, :, 0:HD], in_=ps[:])

                # qT / kT: transposed outputs, weight cols j as stationary
                for j in range(2 * DIM // 128):     # 8 q blocks + 8 k blocks
                    wblk = s1w.tile([128, GD, 128], bf16, tag="wblk")
                    nc.sync.dma_start(
                        out=wblk[:],
                        in_=d_wqkvT.ap()[:, j * 128:(j + 1) * 128]
                        .rearrange("(g p) n -> p g n", p=128))
                    is_q = j < GD
                    src = xTq if is_q else xT
                    width = QP if is_q else S
                    for t in range(width // 1024):
                        ps = bigp.tile([128, 1024], f32, tag="big")
                        for n in range(2):
                            for g in range(GD):
                                nc.tensor.matmul(
                                    out=ps[:, n * 512:(n + 1) * 512],
                                    lhsT=wblk[:, g, :],
                                    rhs=src[:, g, t * 1024 + n * 512:
                                            t * 1024 + (n + 1) * 512],
                                    start=(g == 0), stop=(g == GD - 1))
                        if is_q:
                            dst = qT[:, j, t * 1024:(t + 1) * 1024]
                        else:
                            dst = kT[:, j - GD, t * 1024:(t + 1) * 1024]
                        nc.vector.tensor_copy(out=dst, in_=ps[:])

            # ---------------- stage 2: attention (head pairs) ----------------
            with (
                tc.tile_pool(name="expp", bufs=1) as expp,
                tc.tile_pool(name="nrm", bufs=2) as nrm,
            ):
                for hp in range(HEADS // 2):
                    g = hp
                    eT0 = expp.tile([128, SC, QP], bf16, tag="exp0")
                    eT1 = expp.tile([128, SC, QP], bf16, tag="exp1")
                    # QK for both heads, adjacent matmuls -> PE row groups
                    # 0-63 and 64-127 run concurrently.
                    for kc in range(SC):
                        ps0 = bigp.tile([128, QP], f32, tag="big")
                        ps1 = bigp.tile([128, QP], f32, tag="big")
                        for n in range(2):
                            for p0, ps in ((0, ps0), (HD, ps1)):
                                nc.tensor.matmul(
                                    out=ps[:, n * 512:(n + 1) * 512],
                                    lhsT=kT[p0:p0 + HD, g, kc * 128:(kc + 1) * 128],
                                    rhs=qT[p0:p0 + HD, g, n * 512:(n + 1) * 512],
                                    start=True, stop=True)
                        nc.scalar.activation(out=eT0[:, kc, :], in_=ps0[:], func=Exp)
                        nc.scalar.activation(out=eT1[:, kc, :], in_=ps1[:], func=Exp)
                    for hh, eT in enumerate((eT0, eT1)):
                        h = 2 * hp + hh
                        p0 = hh * HD
                        pv = pvp.tile([HD + 1, QP], f32, tag="pv")
                        for n in range(2):
                            for kc in range(SC):
                                nc.tensor.matmul(
                                    out=pv[:, n * 512:(n + 1) * 512],
                                    lhsT=v_sb[:, kc, h, :],
                                    rhs=eT[:, kc, n * 512:(n + 1) * 512],
                                    start=(kc == 0), stop=(kc == SC - 1))
                        inv = nrm.tile([1, QP], bf16, tag="inv")
                        with nc.allow_low_precision("softmax denom, bf16 ok"):
                            nc.vector.reciprocal(out=inv[:], in_=pv[HD:HD + 1, :])
                        bc = bigp.tile([HD, QP], f32, tag="big")
                        for n in range(2):
                            nc.tensor.matmul(out=bc[:, n * 512:(n + 1) * 512],
                                             lhsT=ones[:],
                                             rhs=inv[:, n * 512:(n + 1) * 512],
                                             start=True, stop=True)
                        raw = nrm.tile([HD, QP], f32, tag="raw")
                        nc.vector.tensor_copy(out=raw[:], in_=pv[0:HD, :])
                        nc.vector.tensor_mul(
                            aoT[p0:p0 + HD, g, :], raw[:], bc[:])

            # ---------------- stage 3: output projection ----------------
            with tc.tile_pool(name="s3sb", bufs=3) as s3sb:
                for m in range(QP // 128):
                    ps = bigp.tile([128, 1024], f32, tag="big")
                    for n in range(2):
                        for g in range(GD):
                            nc.tensor.matmul(
                                out=ps[:, n * 512:(n + 1) * 512],
                                lhsT=aoT[:, g, m * 128:(m + 1) * 128],
                                rhs=woutT[:, g, n * 512:(n + 1) * 512],
                                start=(g == 0), stop=(g == GD - 1))
                    osb = s3sb.tile([128, 1024], f32, tag="osb")
                    nc.vector.tensor_copy(out=osb[:], in_=ps[:])
                    nc.sync.dma_start(
                        out=d_out.ap()[m * 128:(m + 1) * 128, :], in_=osb[:])

    nc.finalize()
    return nc


def kernel(x, mask, Wqkv, Wout, bout):
    from concourse.bass_utils import run_bass_kernel_spmd

    if "nc" not in _CACHE:
        _CACHE["nc"] = _build_program()
    nc = _CACHE["nc"]

    x = np.asarray(x, dtype=np.float32)
    Wqkv = np.asarray(Wqkv, dtype=np.float32)
    Wout = np.asarray(Wout, dtype=np.float32)
    bout = np.asarray(bout, dtype=np.float32)

    wq = Wqkv.copy()
    wq[:DIM] *= 1.0 / np.sqrt(HD)
    wqkvT = np.ascontiguousarray(wq.T).astype(BF16)
    woutT = np.ascontiguousarray(Wout.T).astype(BF16)

    in_maps = []
    for c in range(N_CORES):
        b, half = c // 2, c % 2
        xT = np.ascontiguousarray(x[b].T).astype(BF16)
        in_maps.append({
            "xT": xT,
            "xTq": np.ascontiguousarray(xT[:, half * QP:(half + 1) * QP]),
            "wqkvT": wqkvT,
            "woutT": woutT,
        })

    res = run_bass_kernel_spmd(nc, in_maps, list(range(N_CORES)))
    out = np.empty((B, S, DIM), dtype=np.float32)
    for c in range(N_CORES):
        b, half = c // 2, c % 2
        out[b, half * QP:(half + 1) * QP, :] = res.results[c]["out"]
    out += bout[None, None, :]
    return out
